# revision 1
# baseline (speedup 1.0000x reference)
"""AttentionXL sharded across 8 NeuronCores (tensor parallel over heads).

Contract: kernel(**inputs) takes FULL unsharded inputs, returns FULL output.
Sharding: 16 heads / 8 cores = 2 heads per core. Each core computes its
head-slice of QKV/R projections, relative attention, and a partial output
projection; the host sums the 8 partials (the "all-reduce") and adds bo.
Falls back to a pure-numpy implementation if the device path fails.
"""

import numpy as np

CUR, FULL, BS, DM, H, D = 1024, 2048, 4, 1024, 16, 64
PREV = FULL - CUR
SCALE = 1.0 / D ** 0.5
NC = 8
HL = H // NC  # heads per core

_compiled = None  # cached (pmap_fn,) tuple


def _np_rel_shift(x):
    bs, h, cur, full = x.shape
    xp = np.pad(x, ((0, 0), (0, 0), (0, 0), (1, 0)))
    xp = xp.reshape(bs, h, full + 1, cur)
    return np.ascontiguousarray(xp[:, :, 1:]).reshape(bs, h, cur, full)


def _np_reference(inputs, pos_embedding, full_input, u, v, Wkv, bkv, Wq, bq,
                  Wr, br, Wo, bo, mask):
    cur, bs, _ = inputs.shape
    full = full_input.shape[0]
    kv = full_input.reshape(full * bs, DM) @ Wkv + bkv
    kv = kv.reshape(full, bs, 2 * H * D)
    k, val = kv[..., :H * D], kv[..., H * D:]
    k = k.reshape(full, bs, H, D)
    val = val.reshape(full, bs, H, D)
    q = (inputs.reshape(cur * bs, DM) @ Wq + bq).reshape(cur, bs, H, D)
    r = (pos_embedding @ Wr + br).reshape(full, H, D)
    content = np.einsum('ibhd,jbhd->bhij', q + u, k, optimize=True)
    position = np.einsum('ibhd,jhd->bhij', q + v, r, optimize=True)
    position = _np_rel_shift(position)
    attn = (content + position) * SCALE
    mask_b = np.transpose(mask, (2, 0, 1))[:, None]
    attn = np.where(mask_b, np.float32(-1e20), attn)
    attn = attn - attn.max(axis=-1, keepdims=True)
    np.exp(attn, out=attn)
    attn /= attn.sum(axis=-1, keepdims=True)
    vec = np.einsum('bhij,jbhd->ibhd', attn, val, optimize=True)
    vec = vec.reshape(cur, bs, H * D)
    return (vec.reshape(cur * bs, H * D) @ Wo + bo).reshape(cur, bs, DM).astype(np.float32)


def _build():
    import jax
    import jax.numpy as jnp

    def rel_shift(x):
        bs, h, cur, full = x.shape
        xp = jnp.pad(x, ((0, 0), (0, 0), (0, 0), (1, 0)))
        xp = xp.reshape(bs, h, full + 1, cur)
        return xp[:, :, 1:].reshape(bs, h, cur, full)

    def core_fn(Wq_c, bq_c, Wk_c, bk_c, Wv_c, bv_c, Wr_c, br_c, Wo_c, u_c, v_c,
                inputs, full_input, pos_embedding, mask_b):
        X = inputs.reshape(CUR * BS, DM)
        F = full_input.reshape(FULL * BS, DM)
        q = (X @ Wq_c + bq_c).reshape(CUR, BS, HL, D)
        k = (F @ Wk_c + bk_c).reshape(FULL, BS, HL, D)
        val = (F @ Wv_c + bv_c).reshape(FULL, BS, HL, D)
        r = (pos_embedding @ Wr_c + br_c).reshape(FULL, HL, D)
        content = jnp.einsum('ibhd,jbhd->bhij', q + u_c, k)
        position = jnp.einsum('ibhd,jhd->bhij', q + v_c, r)
        position = rel_shift(position)
        attn = (content + position) * SCALE
        attn = jnp.where(mask_b, -1e20, attn)
        attn = jax.nn.softmax(attn, axis=-1)
        vec = jnp.einsum('bhij,jbhd->ibhd', attn, val)
        vec = vec.reshape(CUR * BS, HL * D)
        return vec @ Wo_c  # partial [CUR*BS, DM]

    pf = jax.pmap(
        core_fn,
        in_axes=(0, 0, 0, 0, 0, 0, 0, 0, 0, 0, 0, None, None, None, None),
    )
    return (pf,)


def _run_sharded(inputs, pos_embedding, full_input, u, v, Wkv, bkv, Wq, bq,
                 Wr, br, Wo, bo, mask):
    global _compiled
    if _compiled is None:
        _compiled = _build()
    (pf,) = _compiled

    W = HL * D  # 128 = per-core head-block width
    Wq_s = np.ascontiguousarray(Wq.reshape(DM, NC, W).transpose(1, 0, 2))
    bq_s = np.ascontiguousarray(bq.reshape(NC, W))
    Wk_s = np.ascontiguousarray(Wkv[:, :H * D].reshape(DM, NC, W).transpose(1, 0, 2))
    bk_s = np.ascontiguousarray(bkv[:H * D].reshape(NC, W))
    Wv_s = np.ascontiguousarray(Wkv[:, H * D:].reshape(DM, NC, W).transpose(1, 0, 2))
    bv_s = np.ascontiguousarray(bkv[H * D:].reshape(NC, W))
    Wr_s = np.ascontiguousarray(Wr.reshape(DM, NC, W).transpose(1, 0, 2))
    br_s = np.ascontiguousarray(br.reshape(NC, W))
    Wo_s = np.ascontiguousarray(Wo.reshape(NC, W, DM))
    u_s = np.ascontiguousarray(u.reshape(NC, HL, D))
    v_s = np.ascontiguousarray(v.reshape(NC, HL, D))
    mask_b = np.ascontiguousarray(np.transpose(mask, (2, 0, 1))[:, None])

    parts = pf(Wq_s, bq_s, Wk_s, bk_s, Wv_s, bv_s, Wr_s, br_s, Wo_s, u_s, v_s,
               inputs, full_input, pos_embedding, mask_b)
    parts = np.asarray(parts, dtype=np.float32)  # [NC, CUR*BS, DM]
    out = parts.sum(axis=0) + bo
    return out.reshape(CUR, BS, DM).astype(np.float32)


def kernel(**inputs):
    try:
        return _run_sharded(**inputs)
    except Exception as e:  # device path unavailable -> correct host fallback
        import traceback
        traceback.print_exc()
        return _np_reference(**inputs)
